# revision 9
# baseline (speedup 1.0000x reference)
"""GAT-style message-passing kernel for Trainium2 (8 NeuronCores, Bass/Tile).

Reference computation (B=8, N=2048):
    a    = softmax(adj, -1); med = lower-median(a); mask = a > med
    w    = (x[:,:,None]*x[:,None,:]) @ W.T + b        # [B,N,N]
    w    = softmax(leaky_relu(w), -1) * mask
    out  = einsum('bi,bij->bj', x, w)

Key identity: w[b,i,k] = x[b,i]*y[b,k] + b[k] with y = x @ W.T (rank-1 +
bias), so no [B,N,N] matmul is needed; everything is fused elementwise
passes plus one weighted reduction:
    out[b,k] = sum_i (x[b,i]/rs[b,i]) * exp(lrelu(x[b,i]*y[b,k]+b[k])) * mask[i,k]
    rs[b,i]  = sum_k exp(lrelu(x[b,i]*y[b,k]+b[k]))

Sharding: rows i are split across the 8 cores (256 rows each, all 8
batches per core).  Each core computes its mask rows from its adj rows,
its shard of y = x@W.T (k-split, AllGather), the partial out over its i
rows, and an AllReduce produces the full output on every core.

The global lower-median of softmax(adj) (a 4M-element order statistic) is
computed on the host and passed in as a scalar; everything O(B*N*N) and
O(N*N) runs on device.
"""

import numpy as np

import concourse.bass as bass  # noqa: F401  (bass types via bacc/tile)
import concourse.mybir as mybir
import concourse.tile as tile
from concourse import bacc
from concourse.bass_utils import run_bass_kernel_spmd

N = 2048
B = 8
NCORES = 8
RPC = N // NCORES  # 256 rows (i) / cols (k) per core
P = 128
ITILES = RPC // P  # 2
NKT = N // 512  # psum-bank sized chunks of the free dim
NEG_SLOPE = 0.01
F32 = mybir.dt.float32
AL = mybir.AluOpType
ACTF = mybir.ActivationFunctionType

# test harness hooks
TRACE = False
LAST_RESULTS = None


def _copy(nc, which, out_ap, in_ap):
    """Alternate PSUM-evacuation copies between ACT and DVE."""
    if which % 2 == 0:
        nc.scalar.copy(out_ap, in_ap)
    else:
        nc.vector.tensor_copy(out_ap, in_ap)


def _kernel(tc, adjc, xcT, xT, wTc, bvec, medv, out):
    nc = tc.nc
    groups = [list(range(NCORES))]

    with (
        tc.tile_pool(name="const", bufs=1) as cpool,
        tc.tile_pool(name="dram", bufs=1, space="DRAM") as dpool,
    ):
        y_bcast = cpool.tile([P, B, N], F32)
        b_bcast = cpool.tile([P, N], F32)
        mask0 = cpool.tile([P, N], F32, tag="mask0")
        mask1 = cpool.tile([P, N], F32, tag="mask1")
        masks = [mask0, mask1]
        xc_sb = cpool.tile([P, ITILES, B], F32)
        med_sb = cpool.tile([P, 1], F32)
        ones1 = cpool.tile([1, P], F32)
        b_row = cpool.tile([1, N], F32)

        nc.sync.dma_start(med_sb[:], medv[:])
        nc.sync.dma_start(b_row[:], bvec[:])
        nc.vector.memset(ones1[:], 1.0)
        nc.sync.dma_start(
            xc_sb[:], xcT[:].rearrange("(it p) b -> p it b", p=P)
        )

        # ---- y shard: y[:, kslice] = x @ W.T[:, kslice], then AllGather ----
        with (
            tc.tile_pool(name="ld", bufs=3) as ldpool,
            tc.tile_pool(name="ps_pre", bufs=1, space="PSUM") as pspre,
        ):
            y_ps = pspre.tile([B, RPC], F32)
            njt = N // P
            for jt in range(njt):
                xT_t = ldpool.tile([P, B], F32, tag="xTt")
                nc.sync.dma_start(xT_t[:], xT[jt * P : (jt + 1) * P, :])
                w_t = ldpool.tile([P, RPC], F32, tag="wt")
                nc.sync.dma_start(w_t[:], wTc[jt * P : (jt + 1) * P, :])
                nc.tensor.matmul(
                    y_ps[:], xT_t[:], w_t[:], start=(jt == 0), stop=(jt == njt - 1)
                )
            y_part = ldpool.tile([B, RPC], F32, tag="ypart", bufs=1)
            nc.scalar.copy(y_part[:], y_ps[:])

            yg_in = dpool.tile([B, RPC], F32)
            yg_out = dpool.tile([NCORES, B, RPC], F32, addr_space="Shared")
            nc.sync.dma_start(yg_in[:], y_part[:])
            nc.gpsimd.collective_compute(
                "AllGather",
                AL.bypass,
                replica_groups=groups,
                ins=[yg_in.opt()],
                outs=[yg_out.opt()],
            )

        # ---- broadcast bias and y rows across partitions via K=1 matmul ----
        with (
            tc.tile_pool(name="ps_bc", bufs=2, space="PSUM") as psbc,
            tc.tile_pool(name="yrow", bufs=2) as ypool,
        ):
            cnt = 0
            for c in range(NKT):
                sl = slice(c * 512, (c + 1) * 512)
                t = psbc.tile([P, 512], F32, tag="bc")
                nc.tensor.matmul(t[:], ones1[:], b_row[:, sl], start=True, stop=True)
                _copy(nc, cnt, b_bcast[:, sl], t[:])
                cnt += 1
            for bb in range(B):
                ytmp = ypool.tile([1, N], F32, tag="ytmp")
                for r in range(NCORES):
                    nc.sync.dma_start(
                        ytmp[:, r * RPC : (r + 1) * RPC], yg_out[r, bb : bb + 1, :]
                    )
                for c in range(NKT):
                    sl = slice(c * 512, (c + 1) * 512)
                    t = psbc.tile([P, 512], F32, tag="bc")
                    nc.tensor.matmul(
                        t[:], ones1[:], ytmp[:, sl], start=True, stop=True
                    )
                    _copy(nc, cnt, y_bcast[:, bb, sl], t[:])
                    cnt += 1

        # ---- mask rows: softmax(adj_rows) > med  (no division needed) ----
        with tc.tile_pool(name="adjp", bufs=2) as apool:
            for it in range(ITILES):
                adj_t = apool.tile([P, N], F32, tag="adj")
                nc.sync.dma_start(adj_t[:], adjc[it * P : (it + 1) * P, :])
                nmax = apool.tile([P, 1], F32, tag="nmax")
                nc.vector.tensor_reduce(
                    nmax[:], adj_t[:], axis=mybir.AxisListType.X, op=AL.max,
                    negate=True,
                )
                eadj = apool.tile([P, N], F32, tag="eadj")
                rs_adj = apool.tile([P, 1], F32, tag="rsadj")
                nc.scalar.activation(
                    eadj[:], adj_t[:], ACTF.Exp, bias=nmax[:], scale=1.0,
                    accum_out=rs_adj[:],
                )
                thr = apool.tile([P, 1], F32, tag="thr")
                nc.vector.tensor_scalar(thr[:], rs_adj[:], med_sb[:], None, AL.mult)
                # mask = (exp(adj-max) > med*rowsum)  <=>  softmax(adj) > med
                nc.vector.tensor_scalar(masks[it][:], eadj[:], thr[:], None, AL.is_gt)

        # ---- main loop: per (batch, i-tile) fused softmax-weighted reduce ----
        with (
            tc.tile_pool(name="main", bufs=2) as mpool,
            tc.tile_pool(name="ps_acc", bufs=2, space="PSUM") as psacc,
        ):
            ar_in = dpool.tile([B, N], F32)
            ar_out = dpool.tile([B, N], F32, addr_space="Shared")
            for bb in range(B):
                acc = psacc.tile([1, N], F32, tag="acc")
                for it in range(ITILES):
                    xcol = xc_sb[:, it, bb : bb + 1]
                    T = mpool.tile([P, N], F32, tag="T")
                    nc.vector.scalar_tensor_tensor(
                        T[:], y_bcast[:, bb, :], xcol, b_bcast[:], AL.mult, AL.add
                    )
                    L = mpool.tile([P, N], F32, tag="L")
                    nc.vector.scalar_tensor_tensor(
                        L[:], T[:], NEG_SLOPE, T[:], AL.mult, AL.max
                    )
                    E = mpool.tile([P, N], F32, tag="E")
                    rs = mpool.tile([P, 1], F32, tag="rs")
                    nc.scalar.activation(E[:], L[:], ACTF.Exp, accum_out=rs[:])
                    EM = mpool.tile([P, N], F32, tag="EM")
                    eng = nc.gpsimd if (bb + it) % 2 == 0 else nc.vector
                    eng.tensor_tensor(EM[:], E[:], masks[it][:], AL.mult)
                    recip = mpool.tile([P, 1], F32, tag="recip")
                    nc.vector.reciprocal(recip[:], rs[:])
                    coeff = mpool.tile([P, 1], F32, tag="coeff")
                    nc.vector.tensor_scalar(coeff[:], recip[:], xcol, None, AL.mult)
                    for c in range(NKT):
                        sl = slice(c * 512, (c + 1) * 512)
                        nc.tensor.matmul(
                            acc[:, sl], coeff[:], EM[:, sl],
                            start=(it == 0), stop=(it == ITILES - 1),
                        )
                orow = mpool.tile([1, N], F32, tag="orow")
                _copy(nc, bb, orow[:], acc[:])
                nc.sync.dma_start(ar_in[bb : bb + 1, :], orow[:])

            # ---- AllReduce partial outputs over i-shards ----
            nc.gpsimd.collective_compute(
                "AllReduce",
                AL.add,
                replica_groups=groups,
                ins=[ar_in.opt()],
                outs=[ar_out.opt()],
            )
            nc.sync.dma_start(out[:], ar_out[:])


def build_nc():
    nc = bacc.Bacc("TRN2", target_bir_lowering=False, num_devices=NCORES)
    adjc = nc.dram_tensor("adjc", [RPC, N], F32, kind="ExternalInput")
    xcT = nc.dram_tensor("xcT", [RPC, B], F32, kind="ExternalInput")
    xT = nc.dram_tensor("xT", [N, B], F32, kind="ExternalInput")
    wTc = nc.dram_tensor("wTc", [N, RPC], F32, kind="ExternalInput")
    bvec = nc.dram_tensor("bvec", [1, N], F32, kind="ExternalInput")
    medv = nc.dram_tensor("medv", [P, 1], F32, kind="ExternalInput")
    out = nc.dram_tensor("out", [B, N], F32, kind="ExternalOutput")
    with tile.TileContext(nc) as tc:
        _kernel(tc, adjc, xcT, xT, wTc, bvec, medv, out)
    nc.compile()
    return nc


def host_median(adj):
    """Lower median of softmax(adj, -1), float32, matching torch.median."""
    adj = np.asarray(adj, np.float32)
    m = adj.max(axis=1, keepdims=True)
    e = np.exp(adj - m, dtype=np.float32)
    a = (e / e.sum(axis=1, keepdims=True, dtype=np.float32)).astype(np.float32)
    flat = a.reshape(-1)
    kth = (flat.size - 1) // 2
    return np.partition(flat, kth)[kth]


def prepare_inputs(x, adj, W, b):
    x = np.ascontiguousarray(np.asarray(x, np.float32))
    adj = np.ascontiguousarray(np.asarray(adj, np.float32))
    W = np.ascontiguousarray(np.asarray(W, np.float32))
    b = np.ascontiguousarray(np.asarray(b, np.float32))
    med = host_median(adj)
    WT = np.ascontiguousarray(W.T)
    xT = np.ascontiguousarray(x.T)
    medv = np.full((P, 1), med, np.float32)
    bvec = np.ascontiguousarray(b.reshape(1, N))
    in_maps = []
    for c in range(NCORES):
        sl = slice(c * RPC, (c + 1) * RPC)
        in_maps.append(
            {
                "adjc": np.ascontiguousarray(adj[sl]),
                "xcT": np.ascontiguousarray(xT[sl]),
                "xT": xT,
                "wTc": np.ascontiguousarray(WT[:, sl]),
                "bvec": bvec,
                "medv": medv,
            }
        )
    return in_maps


_NC_CACHE = None


def kernel(x, adj, W, b):
    global _NC_CACHE, LAST_RESULTS
    if _NC_CACHE is None:
        _NC_CACHE = build_nc()
    in_maps = prepare_inputs(x, adj, W, b)
    res = run_bass_kernel_spmd(
        _NC_CACHE, in_maps, core_ids=list(range(NCORES)), trace=TRACE
    )
    LAST_RESULTS = res
    return np.asarray(res.results[0]["out"], np.float32)


# revision 13
# speedup vs baseline: 1.2675x; 1.2675x over previous
"""GAT-style message-passing kernel for Trainium2 (8 NeuronCores, Bass/Tile).

Reference computation (B=8, N=2048):
    a    = softmax(adj, -1); med = lower-median(a); mask = a > med
    w    = (x[:,:,None]*x[:,None,:]) @ W.T + b        # [B,N,N]
    w    = softmax(leaky_relu(w), -1) * mask
    out  = einsum('bi,bij->bj', x, w)

Key identity: w[b,i,k] = x[b,i]*y[b,k] + b[k] with y = x @ W.T (rank-1 +
bias), so no [B,N,N] matmul is needed; everything is fused elementwise
passes plus one weighted reduction:
    out[b,k] = sum_i (x[b,i]/rs[b,i]) * exp(lrelu(x[b,i]*y[b,k]+b[k])) * mask[i,k]
    rs[b,i]  = sum_k exp(lrelu(x[b,i]*y[b,k]+b[k]))

Sharding: rows i are split across the 8 cores (256 rows each, all 8
batches per core).  Each core computes its mask rows from its adj rows,
its shard of y = x@W.T (k-split, AllGather), the partial out over its i
rows, and an AllReduce produces the full output on every core.

The global lower-median of softmax(adj) (a 4M-element order statistic) is
computed on the host and passed in as a scalar; everything O(B*N*N) and
O(N*N) runs on device.
"""

import numpy as np

import concourse.bass as bass  # noqa: F401  (bass types via bacc/tile)
import concourse.mybir as mybir
import concourse.tile as tile
from concourse import bacc
from concourse.bass_utils import run_bass_kernel_spmd

N = 2048
B = 8
NCORES = 8
RPC = N // NCORES  # 256 rows (i) / cols (k) per core
P = 128
ITILES = RPC // P  # 2
NJT = N // P  # 16 j-tiles for the y matmul
NKT = N // 512  # psum-bank sized chunks of the free dim
NEG_SLOPE = 0.01
F32 = mybir.dt.float32
BF16 = mybir.dt.bfloat16
AL = mybir.AluOpType
ACTF = mybir.ActivationFunctionType

# test harness hooks
TRACE = False
LAST_RESULTS = None


def _kernel(tc, adjc, xcT, xT, wTc, bvec, medv, out, sim_compat):
    nc = tc.nc
    groups = [list(range(NCORES))]

    with (
        tc.tile_pool(name="const", bufs=1) as cpool,
        tc.tile_pool(name="dram", bufs=1, space="DRAM") as dpool,
    ):
        y_bcast = cpool.tile([P, B, N], F32)
        b_bcast = cpool.tile([P, N], F32)
        mask0 = cpool.tile([P, N], F32, tag="mask0")
        mask1 = cpool.tile([P, N], F32, tag="mask1")
        masks = [mask0, mask1]
        xc_sb = cpool.tile([P, ITILES, B], F32)
        med_sb = cpool.tile([P, 1], F32)

        b_row = cpool.tile([1, N], F32)
        nc.sync.dma_start(med_sb[:], medv[:])
        nc.sync.dma_start(xc_sb[:], xcT[:].rearrange("(it p) b -> p it b", p=P))
        nc.sync.dma_start(b_row[:], bvec[:])
        nc.gpsimd.partition_broadcast(b_bcast[:], b_row[:])

        # ---- y shard: y[:, kslice] = x @ W.T[:, kslice], then AllGather ----
        with (
            tc.tile_pool(name="ld", bufs=1) as ldpool,
            tc.tile_pool(name="ps_pre", bufs=1, space="PSUM") as pspre,
        ):
            xT_t = ldpool.tile([P, NJT, B], F32)
            nc.sync.dma_start(xT_t[:], xT[:].rearrange("(jt p) b -> p jt b", p=P))
            w_t = ldpool.tile([P, NJT, RPC], F32)
            nc.sync.dma_start(w_t[:], wTc[:].rearrange("(jt p) k -> p jt k", p=P))
            y_ps = pspre.tile([B, RPC], F32)
            for jt in range(NJT):
                nc.tensor.matmul(
                    y_ps[:], xT_t[:, jt, :], w_t[:, jt, :],
                    start=(jt == 0), stop=(jt == NJT - 1),
                )
            y_part = ldpool.tile([B, RPC], F32)
            nc.scalar.copy(y_part[:], y_ps[:])

            yg_in = dpool.tile([B, RPC], F32)
            yg_out = dpool.tile([NCORES, B, RPC], F32, addr_space="Shared")
            nc.sync.dma_start(yg_in[:], y_part[:])
            nc.gpsimd.collective_compute(
                "AllGather",
                AL.bypass,
                replica_groups=groups,
                ins=[yg_in.opt()],
                outs=[yg_out.opt()],
            )
            # broadcast y[b, :] (8 strided chunks in the gather buffer) to
            # all partitions, one DMA broadcast per batch
            yg_bview = yg_out[:].rearrange("r b k -> b r k")
            for bb in range(B):
                ytmp = ldpool.tile([1, NCORES, RPC], F32, tag="ytmp", bufs=2)
                nc.sync.dma_start(ytmp[:], yg_bview[bb : bb + 1])
                nc.gpsimd.partition_broadcast(y_bcast[:, bb, :], ytmp[:])

        # ---- mask rows: softmax(adj_rows) > med  (no division needed) ----
        with tc.tile_pool(name="adjp", bufs=2) as apool:
            for it in range(ITILES):
                adj_t = apool.tile([P, N], F32, tag="adj")
                nc.sync.dma_start(adj_t[:], adjc[it * P : (it + 1) * P, :])
                nmax = apool.tile([P, 1], F32, tag="nmax")
                nc.vector.tensor_reduce(
                    nmax[:], adj_t[:], axis=mybir.AxisListType.X, op=AL.max,
                    negate=True,
                )
                eadj = apool.tile([P, N], F32, tag="eadj")
                rs_adj = apool.tile([P, 1], F32, tag="rsadj")
                nc.scalar.activation(
                    eadj[:], adj_t[:], ACTF.Exp, bias=nmax[:], scale=1.0,
                    accum_out=rs_adj[:],
                )
                thr = apool.tile([P, 1], F32, tag="thr")
                nc.vector.tensor_scalar(thr[:], rs_adj[:], med_sb[:], None, AL.mult)
                # mask = (exp(adj-max) > med*rowsum)  <=>  softmax(adj) > med
                nc.vector.tensor_scalar(masks[it][:], eadj[:], thr[:], None, AL.is_gt)

        # ---- main loop: per (batch, i-tile) fused softmax-weighted reduce ----
        with (
            tc.tile_pool(name="main", bufs=3) as mpool,
            tc.tile_pool(name="ps_acc", bufs=2, space="PSUM") as psacc,
        ):
            ar_in = dpool.tile([B, N], F32)
            ar_out = dpool.tile([B, N], F32, addr_space="Shared")
            for bb in range(B):
                acc = psacc.tile([1, N], F32, tag="acc")
                for it in range(ITILES):
                    xcol = xc_sb[:, it, bb : bb + 1]
                    T = mpool.tile([P, N], F32, tag="T")
                    nc.vector.scalar_tensor_tensor(
                        T[:], y_bcast[:, bb, :], xcol, b_bcast[:], AL.mult, AL.add
                    )
                    L = mpool.tile([P, N], F32, tag="L")
                    if sim_compat:
                        nc.vector.scalar_tensor_tensor(
                            L[:], T[:], NEG_SLOPE, T[:], AL.mult, AL.max
                        )
                    else:
                        nc.scalar.activation(
                            L[:], T[:], ACTF.Lrelu, alpha=NEG_SLOPE
                        )
                    E = mpool.tile([P, N], F32, tag="E")
                    rs = mpool.tile([P, 1], F32, tag="rs")
                    nc.scalar.activation(E[:], L[:], ACTF.Exp, accum_out=rs[:])
                    EM = mpool.tile([P, N], BF16, tag="EM")
                    nc.vector.tensor_tensor(EM[:], E[:], masks[it][:], AL.mult)
                    recip = mpool.tile([P, 1], F32, tag="recip")
                    nc.vector.reciprocal(recip[:], rs[:])
                    coeff = mpool.tile([P, 1], BF16, tag="coeff")
                    nc.vector.tensor_scalar(coeff[:], recip[:], xcol, None, AL.mult)
                    for c in range(NKT):
                        sl = slice(c * 512, (c + 1) * 512)
                        nc.tensor.matmul(
                            acc[:, sl], coeff[:], EM[:, sl],
                            start=(it == 0), stop=(it == ITILES - 1),
                        )
                orow = mpool.tile([1, N], F32, tag="orow")
                if bb % 2 == 0:
                    nc.scalar.copy(orow[:], acc[:])
                else:
                    nc.vector.tensor_copy(orow[:], acc[:])
                nc.sync.dma_start(ar_in[bb : bb + 1, :], orow[:])

            # ---- AllReduce partial outputs over i-shards ----
            nc.gpsimd.collective_compute(
                "AllReduce",
                AL.add,
                replica_groups=groups,
                ins=[ar_in.opt()],
                outs=[ar_out.opt()],
            )
            nc.sync.dma_start(out[:], ar_out[:])


def build_nc(sim_compat=False):
    nc = bacc.Bacc("TRN2", target_bir_lowering=False, num_devices=NCORES)
    adjc = nc.dram_tensor("adjc", [RPC, N], F32, kind="ExternalInput")
    xcT = nc.dram_tensor("xcT", [RPC, B], F32, kind="ExternalInput")
    xT = nc.dram_tensor("xT", [N, B], F32, kind="ExternalInput")
    wTc = nc.dram_tensor("wTc", [N, RPC], F32, kind="ExternalInput")
    bvec = nc.dram_tensor("bvec", [1, N], F32, kind="ExternalInput")
    medv = nc.dram_tensor("medv", [P, 1], F32, kind="ExternalInput")
    out = nc.dram_tensor("out", [B, N], F32, kind="ExternalOutput")
    with tile.TileContext(nc) as tc:
        _kernel(tc, adjc, xcT, xT, wTc, bvec, medv, out, sim_compat)
    nc.compile()
    return nc


def host_median(adj):
    """Lower median of softmax(adj, -1), float32, matching torch.median."""
    adj = np.asarray(adj, np.float32)
    m = adj.max(axis=1, keepdims=True)
    e = np.exp(adj - m, dtype=np.float32)
    a = (e / e.sum(axis=1, keepdims=True, dtype=np.float32)).astype(np.float32)
    flat = a.reshape(-1)
    kth = (flat.size - 1) // 2
    return np.partition(flat, kth)[kth]


def prepare_inputs(x, adj, W, b):
    x = np.ascontiguousarray(np.asarray(x, np.float32))
    adj = np.ascontiguousarray(np.asarray(adj, np.float32))
    W = np.ascontiguousarray(np.asarray(W, np.float32))
    b = np.ascontiguousarray(np.asarray(b, np.float32))
    med = host_median(adj)
    WT = np.ascontiguousarray(W.T)
    xT = np.ascontiguousarray(x.T)
    medv = np.full((P, 1), med, np.float32)
    bvec = np.ascontiguousarray(b.reshape(1, N))
    in_maps = []
    for c in range(NCORES):
        sl = slice(c * RPC, (c + 1) * RPC)
        in_maps.append(
            {
                "adjc": np.ascontiguousarray(adj[sl]),
                "xcT": np.ascontiguousarray(xT[sl]),
                "xT": xT,
                "wTc": np.ascontiguousarray(WT[:, sl]),
                "bvec": bvec,
                "medv": medv,
            }
        )
    return in_maps


_NC_CACHE = None


def kernel(x, adj, W, b):
    global _NC_CACHE, LAST_RESULTS
    if _NC_CACHE is None:
        _NC_CACHE = build_nc()
    in_maps = prepare_inputs(x, adj, W, b)
    res = run_bass_kernel_spmd(
        _NC_CACHE, in_maps, core_ids=list(range(NCORES)), trace=TRACE
    )
    LAST_RESULTS = res
    return np.asarray(res.results[0]["out"], np.float32)


# revision 17
# speedup vs baseline: 1.3679x; 1.0792x over previous
"""GAT-style message-passing kernel for Trainium2 (8 NeuronCores, Bass/Tile).

Reference computation (B=8, N=2048):
    a    = softmax(adj, -1); med = lower-median(a); mask = a > med
    w    = (x[:,:,None]*x[:,None,:]) @ W.T + b        # [B,N,N]
    w    = softmax(leaky_relu(w), -1) * mask
    out  = einsum('bi,bij->bj', x, w)

Key identity: w[b,i,k] = x[b,i]*y[b,k] + b[k] with y = x @ W.T (rank-1 +
bias), so no [B,N,N] matmul is needed; everything is fused elementwise
passes plus one weighted reduction:
    out[b,k] = sum_i (x[b,i]/rs[b,i]) * exp(lrelu(x[b,i]*y[b,k]+b[k])) * mask[i,k]
    rs[b,i]  = sum_k exp(lrelu(x[b,i]*y[b,k]+b[k]))

Sharding: rows i are split across the 8 cores (256 rows each, all 8
batches per core).  Each core computes its mask rows from its adj rows,
its shard of y = x@W.T (k-split, AllGather), the partial out over its i
rows, and an AllReduce produces the full output on every core.

The global lower-median of softmax(adj) (a 4M-element order statistic) is
computed on the host and passed in as a scalar; everything O(B*N*N) and
O(N*N) runs on device.
"""

import numpy as np

import concourse.bass as bass  # noqa: F401  (bass types via bacc/tile)
import concourse.mybir as mybir
import concourse.tile as tile
from concourse import bacc
from concourse.bass_utils import run_bass_kernel_spmd

N = 2048
B = 8
NCORES = 8
RPC = N // NCORES  # 256 rows (i) / cols (k) per core
P = 128
ITILES = RPC // P  # 2
NJT = N // P  # 16 j-tiles for the y matmul
NKT = N // 512  # psum-bank sized chunks of the free dim
NEG_SLOPE = 0.01
F32 = mybir.dt.float32
BF16 = mybir.dt.bfloat16
AL = mybir.AluOpType
ACTF = mybir.ActivationFunctionType

# test harness hooks
TRACE = False
LAST_RESULTS = None


def _kernel(tc, adjc, xcT, xT, wTc, bvec, medv, out, sim_compat):
    nc = tc.nc
    groups = [list(range(NCORES))]

    with (
        tc.tile_pool(name="const", bufs=1) as cpool,
        tc.tile_pool(name="dram", bufs=1, space="DRAM") as dpool,
    ):
        y_bcast = cpool.tile([P, B, N], F32)
        b_bcast = cpool.tile([P, N], F32)
        mask0 = cpool.tile([P, N], F32, tag="mask0")
        mask1 = cpool.tile([P, N], F32, tag="mask1")
        masks = [mask0, mask1]
        xc_sb = cpool.tile([P, ITILES, B], F32)
        med_sb = cpool.tile([P, 1], F32)

        b_row = cpool.tile([1, N], F32)
        nc.sync.dma_start(med_sb[:], medv[:])
        nc.sync.dma_start(xc_sb[:], xcT[:].rearrange("(it p) b -> p it b", p=P))
        nc.sync.dma_start(b_row[:], bvec[:])
        nc.gpsimd.partition_broadcast(b_bcast[:], b_row[:])

        # ---- mask rows: softmax(adj_rows) > med  (no division needed) ----
        with tc.tile_pool(name="adjp", bufs=2) as apool:
            for it in range(ITILES):
                adj_t = apool.tile([P, N], F32, tag="adj")
                nc.sync.dma_start(adj_t[:], adjc[it * P : (it + 1) * P, :])
                nmax = apool.tile([P, 1], F32, tag="nmax")
                nc.vector.tensor_reduce(
                    nmax[:], adj_t[:], axis=mybir.AxisListType.X, op=AL.max,
                    negate=True,
                )
                eadj = apool.tile([P, N], F32, tag="eadj")
                rs_adj = apool.tile([P, 1], F32, tag="rsadj")
                nc.scalar.activation(
                    eadj[:], adj_t[:], ACTF.Exp, bias=nmax[:], scale=1.0,
                    accum_out=rs_adj[:],
                )
                thr = apool.tile([P, 1], F32, tag="thr")
                nc.vector.tensor_scalar(thr[:], rs_adj[:], med_sb[:], None, AL.mult)
                # mask = (exp(adj-max) > med*rowsum)  <=>  softmax(adj) > med
                nc.vector.tensor_scalar(masks[it][:], eadj[:], thr[:], None, AL.is_gt)

        # ---- y shard: y[:, kslice] = x @ W.T[:, kslice], then AllGather ----
        with (
            tc.tile_pool(name="ld", bufs=1) as ldpool,
            tc.tile_pool(name="ps_pre", bufs=1, space="PSUM") as pspre,
        ):
            xT_t = ldpool.tile([P, NJT, B], F32)
            nc.sync.dma_start(xT_t[:], xT[:].rearrange("(jt p) b -> p jt b", p=P))
            w_t = ldpool.tile([P, NJT, RPC], F32)
            nc.sync.dma_start(w_t[:], wTc[:].rearrange("(jt p) k -> p jt k", p=P))
            y_ps = pspre.tile([B, RPC], F32)
            for jt in range(NJT):
                nc.tensor.matmul(
                    y_ps[:], xT_t[:, jt, :], w_t[:, jt, :],
                    start=(jt == 0), stop=(jt == NJT - 1),
                )
            y_part = ldpool.tile([B, RPC], F32)
            nc.scalar.copy(y_part[:], y_ps[:])

            yg_in = dpool.tile([B, RPC], F32)
            yg_out = dpool.tile([NCORES, B, RPC], F32, addr_space="Shared")
            nc.sync.dma_start(yg_in[:], y_part[:])
            nc.gpsimd.collective_compute(
                "AllGather",
                AL.bypass,
                replica_groups=groups,
                ins=[yg_in.opt()],
                outs=[yg_out.opt()],
            )
            # broadcast y[b, :] (8 strided chunks in the gather buffer) to
            # all partitions, one DMA broadcast per batch
            yg_bview = yg_out[:].rearrange("r b k -> b r k")
            for bb in range(B):
                ytmp = ldpool.tile([1, NCORES, RPC], F32, tag="ytmp", bufs=2)
                nc.gpsimd.dma_start(ytmp[:], yg_bview[bb : bb + 1])
                nc.gpsimd.partition_broadcast(y_bcast[:, bb, :], ytmp[:])

        # ---- main loop: groups of 4 (2 batches x 2 i-tiles), fused ----
        # grouping keeps the ACT engine on one function table at a time
        with (
            tc.tile_pool(name="main", bufs=6) as mpool,
            tc.tile_pool(name="ps_acc", bufs=2, space="PSUM") as psacc,
        ):
            ar_in = dpool.tile([B, N], F32)
            ar_out = dpool.tile([B, N], F32, addr_space="Shared")
            for bp in range(B // 2):
                bbs = (2 * bp, 2 * bp + 1)
                quad = [(bb, it) for bb in bbs for it in range(ITILES)]
                accs = {}
                for bb in bbs:
                    accs[bb] = psacc.tile([1, N], F32, tag="acc", name=f"acc{bb}")
                tiles = {}
                for bb, it in quad:
                    xcol = xc_sb[:, it, bb : bb + 1]
                    T = mpool.tile([P, N], F32, tag="T")
                    nc.vector.scalar_tensor_tensor(
                        T[:], y_bcast[:, bb, :], xcol, b_bcast[:], AL.mult, AL.add
                    )
                    tiles[bb, it] = T
                if sim_compat:
                    for bb, it in quad:
                        T = tiles[bb, it]
                        nc.vector.scalar_tensor_tensor(
                            T[:], T[:], NEG_SLOPE, T[:], AL.mult, AL.max
                        )
                else:
                    for bb, it in quad:
                        T = tiles[bb, it]
                        nc.scalar.activation(T[:], T[:], ACTF.Lrelu, alpha=NEG_SLOPE)
                for bb, it in quad:
                    T = tiles[bb, it]
                    rs = mpool.tile([P, 1], F32, tag="rs")
                    nc.scalar.activation(T[:], T[:], ACTF.Exp, accum_out=rs[:])
                    tiles["rs", bb, it] = rs
                for bb, it in quad:
                    T = tiles[bb, it]
                    rs = tiles["rs", bb, it]
                    xcol = xc_sb[:, it, bb : bb + 1]
                    EM = mpool.tile([P, N], BF16, tag="EM", bufs=4)
                    nc.vector.tensor_tensor(EM[:], T[:], masks[it][:], AL.mult)
                    recip = mpool.tile([P, 1], F32, tag="recip")
                    nc.vector.reciprocal(recip[:], rs[:])
                    coeff = mpool.tile([P, 1], BF16, tag="coeff")
                    nc.vector.tensor_scalar(coeff[:], recip[:], xcol, None, AL.mult)
                    for c in range(NKT):
                        sl = slice(c * 512, (c + 1) * 512)
                        nc.tensor.matmul(
                            accs[bb][:, sl], coeff[:], EM[:, sl],
                            start=(it == 0), stop=(it == ITILES - 1),
                        )
                for bb in bbs:
                    orow = mpool.tile([1, N], F32, tag="orow", bufs=2)
                    nc.scalar.copy(orow[:], accs[bb][:])
                    nc.sync.dma_start(ar_in[bb : bb + 1, :], orow[:])

            # ---- AllReduce partial outputs over i-shards ----
            nc.gpsimd.collective_compute(
                "AllReduce",
                AL.add,
                replica_groups=groups,
                ins=[ar_in.opt()],
                outs=[ar_out.opt()],
            )
            nc.sync.dma_start(out[:], ar_out[:])


def build_nc(sim_compat=False):
    nc = bacc.Bacc("TRN2", target_bir_lowering=False, num_devices=NCORES)
    adjc = nc.dram_tensor("adjc", [RPC, N], F32, kind="ExternalInput")
    xcT = nc.dram_tensor("xcT", [RPC, B], F32, kind="ExternalInput")
    xT = nc.dram_tensor("xT", [N, B], F32, kind="ExternalInput")
    wTc = nc.dram_tensor("wTc", [N, RPC], F32, kind="ExternalInput")
    bvec = nc.dram_tensor("bvec", [1, N], F32, kind="ExternalInput")
    medv = nc.dram_tensor("medv", [P, 1], F32, kind="ExternalInput")
    out = nc.dram_tensor("out", [B, N], F32, kind="ExternalOutput")
    with tile.TileContext(nc) as tc:
        _kernel(tc, adjc, xcT, xT, wTc, bvec, medv, out, sim_compat)
    nc.compile()
    return nc


def host_median(adj):
    """Lower median of softmax(adj, -1), float32, matching torch.median."""
    adj = np.asarray(adj, np.float32)
    m = adj.max(axis=1, keepdims=True)
    e = np.exp(adj - m, dtype=np.float32)
    a = (e / e.sum(axis=1, keepdims=True, dtype=np.float32)).astype(np.float32)
    flat = a.reshape(-1)
    kth = (flat.size - 1) // 2
    return np.partition(flat, kth)[kth]


def prepare_inputs(x, adj, W, b):
    x = np.ascontiguousarray(np.asarray(x, np.float32))
    adj = np.ascontiguousarray(np.asarray(adj, np.float32))
    W = np.ascontiguousarray(np.asarray(W, np.float32))
    b = np.ascontiguousarray(np.asarray(b, np.float32))
    med = host_median(adj)
    WT = np.ascontiguousarray(W.T)
    xT = np.ascontiguousarray(x.T)
    medv = np.full((P, 1), med, np.float32)
    bvec = np.ascontiguousarray(b.reshape(1, N))
    in_maps = []
    for c in range(NCORES):
        sl = slice(c * RPC, (c + 1) * RPC)
        in_maps.append(
            {
                "adjc": np.ascontiguousarray(adj[sl]),
                "xcT": np.ascontiguousarray(xT[sl]),
                "xT": xT,
                "wTc": np.ascontiguousarray(WT[:, sl]),
                "bvec": bvec,
                "medv": medv,
            }
        )
    return in_maps


_NC_CACHE = None


def kernel(x, adj, W, b):
    global _NC_CACHE, LAST_RESULTS
    if _NC_CACHE is None:
        _NC_CACHE = build_nc()
    in_maps = prepare_inputs(x, adj, W, b)
    res = run_bass_kernel_spmd(
        _NC_CACHE, in_maps, core_ids=list(range(NCORES)), trace=TRACE
    )
    LAST_RESULTS = res
    return np.asarray(res.results[0]["out"], np.float32)


# revision 22
# speedup vs baseline: 1.4624x; 1.0691x over previous
"""GAT-style message-passing kernel for Trainium2 (8 NeuronCores, Bass/Tile).

Reference computation (B=8, N=2048):
    a    = softmax(adj, -1); med = lower-median(a); mask = a > med
    w    = (x[:,:,None]*x[:,None,:]) @ W.T + b        # [B,N,N]
    w    = softmax(leaky_relu(w), -1) * mask
    out  = einsum('bi,bij->bj', x, w)

Key identity: w[b,i,k] = x[b,i]*y[b,k] + b[k] with y = x @ W.T (rank-1 +
bias), so no [B,N,N] matmul is needed; everything is fused elementwise
passes plus one weighted reduction:
    out[b,k] = sum_i (x[b,i]/rs[b,i]) * exp(lrelu(x[b,i]*y[b,k]+b[k])) * mask[i,k]
    rs[b,i]  = sum_k exp(lrelu(x[b,i]*y[b,k]+b[k]))

Sharding: rows i are split across the 8 cores (256 rows each, all 8
batches per core).  Each core computes its mask rows from its adj rows,
its shard of y = x@W.T (k-split, AllGather), the partial out over its i
rows, and an AllReduce produces the full output on every core.

The global lower-median of softmax(adj) (a 4M-element order statistic) is
computed on the host and passed in as a scalar; everything O(B*N*N) and
O(N*N) runs on device.
"""

import numpy as np

import concourse.bass as bass  # noqa: F401  (bass types via bacc/tile)
import concourse.mybir as mybir
import concourse.tile as tile
from concourse import bacc
from concourse.bass_utils import run_bass_kernel_spmd

N = 2048
B = 8
NCORES = 8
RPC = N // NCORES  # 256 rows (i) / cols (k) per core
P = 128
ITILES = RPC // P  # 2
NJT = N // P  # 16 j-tiles for the y matmul
NKT = N // 512  # psum-bank sized chunks of the free dim
NEG_SLOPE = 0.01
F32 = mybir.dt.float32
BF16 = mybir.dt.bfloat16
AL = mybir.AluOpType
ACTF = mybir.ActivationFunctionType

# test harness hooks
TRACE = False
LAST_RESULTS = None


def _kernel(tc, adjc, xcT, xT, wTc, bvec, medv, out, sim_compat):
    nc = tc.nc
    groups = [list(range(NCORES))]

    with (
        tc.tile_pool(name="const", bufs=1) as cpool,
        tc.tile_pool(name="dram", bufs=1, space="DRAM") as dpool,
    ):
        y_bcast = [
            cpool.tile([P, N], F32, tag=f"ybc{bb}", name=f"ybc{bb}") for bb in range(B)
        ]
        b_bcast = cpool.tile([P, N], F32)
        mask0 = cpool.tile([P, N], BF16, tag="mask0")
        mask1 = cpool.tile([P, N], BF16, tag="mask1")
        masks = [mask0, mask1]
        xc_sb = cpool.tile([P, ITILES, B], F32)
        med_sb = cpool.tile([P, 1], F32)

        b_row = cpool.tile([1, N], F32)
        nc.sync.dma_start(med_sb[:], medv[:])
        nc.sync.dma_start(xc_sb[:], xcT[:].rearrange("(it p) b -> p it b", p=P))
        nc.sync.dma_start(b_row[:], bvec[:])
        nc.gpsimd.partition_broadcast(b_bcast[:], b_row[:])

        # ---- mask rows: softmax(adj_rows) > med  (no division needed) ----
        with tc.tile_pool(name="adjp", bufs=2) as apool:
            for it in range(ITILES):
                adj_t = apool.tile([P, N], F32, tag="adj")
                nc.sync.dma_start(adj_t[:], adjc[it * P : (it + 1) * P, :])
                nmax = apool.tile([P, 1], F32, tag="nmax")
                nc.vector.tensor_reduce(
                    nmax[:], adj_t[:], axis=mybir.AxisListType.X, op=AL.max,
                    negate=True,
                )
                eadj = apool.tile([P, N], F32, tag="eadj")
                rs_adj = apool.tile([P, 1], F32, tag="rsadj")
                nc.scalar.activation(
                    eadj[:], adj_t[:], ACTF.Exp, bias=nmax[:], scale=1.0,
                    accum_out=rs_adj[:],
                )
                thr = apool.tile([P, 1], F32, tag="thr")
                nc.vector.tensor_scalar(thr[:], rs_adj[:], med_sb[:], None, AL.mult)
                # mask = (exp(adj-max) > med*rowsum)  <=>  softmax(adj) > med
                nc.vector.tensor_scalar(masks[it][:], eadj[:], thr[:], None, AL.is_gt)

        # ---- y shard: y[:, kslice] = x @ W.T[:, kslice], then AllGather ----
        with (
            tc.tile_pool(name="ld", bufs=1) as ldpool,
            tc.tile_pool(name="ps_pre", bufs=1, space="PSUM") as pspre,
        ):
            xT_t = ldpool.tile([P, NJT, B], F32)
            nc.sync.dma_start(xT_t[:], xT[:].rearrange("(jt p) b -> p jt b", p=P))
            w_t = ldpool.tile([P, NJT, RPC], F32)
            nc.sync.dma_start(w_t[:], wTc[:].rearrange("(jt p) k -> p jt k", p=P))
            y_ps = pspre.tile([B, RPC], F32)
            for jt in range(NJT):
                nc.tensor.matmul(
                    y_ps[:], xT_t[:, jt, :], w_t[:, jt, :],
                    start=(jt == 0), stop=(jt == NJT - 1),
                )
            y_part = ldpool.tile([B, RPC], F32)
            nc.scalar.copy(y_part[:], y_ps[:])

            yg_in = dpool.tile([B, RPC], F32)
            yg_out = dpool.tile([NCORES, B, RPC], F32, addr_space="Shared")
            nc.sync.dma_start(yg_in[:], y_part[:])
            nc.gpsimd.collective_compute(
                "AllGather",
                AL.bypass,
                replica_groups=groups,
                ins=[yg_in.opt()],
                outs=[yg_out.opt()],
            )
            # broadcast y[b, :] (8 strided chunks in the gather buffer) to
            # all partitions, one DMA broadcast per batch
            yg_bview = yg_out[:].rearrange("r b k -> b r k")
            ytmps = []
            for bb in range(B):
                ytmp = ldpool.tile(
                    [1, NCORES, RPC], F32, tag=f"ytmp{bb}", name=f"ytmp{bb}"
                )
                nc.sync.dma_start(ytmp[:], yg_bview[bb : bb + 1])
                ytmps.append(ytmp)
            for bb in range(B):
                nc.gpsimd.partition_broadcast(y_bcast[bb][:], ytmps[bb][:])

        # ---- main loop: groups of 4 (2 batches x 2 i-tiles), fused ----
        # grouping keeps the ACT engine on one function table at a time
        with (
            tc.tile_pool(name="main", bufs=6) as mpool,
            tc.tile_pool(name="ps_acc", bufs=2, space="PSUM") as psacc,
        ):
            ar_in = dpool.tile([B, N], F32)
            ar_out0 = dpool.tile([B // 2, N], F32, addr_space="Shared")
            ar_out1 = dpool.tile([B // 2, N], F32, addr_space="Shared")
            for bp in range(B // 2):
                bbs = (2 * bp, 2 * bp + 1)
                quad = [(bb, it) for bb in bbs for it in range(ITILES)]
                accs = {}
                for bb in bbs:
                    accs[bb] = psacc.tile([1, N], F32, tag="acc", name=f"acc{bb}")
                tiles = {}
                for bb, it in quad:
                    xcol = xc_sb[:, it, bb : bb + 1]
                    T = mpool.tile([P, N], F32, tag="T")
                    nc.vector.scalar_tensor_tensor(
                        T[:], y_bcast[bb][:], xcol, b_bcast[:], AL.mult, AL.add
                    )
                    tiles[bb, it] = T
                # lrelu: first half of the quad on ACT (one table load),
                # second half on DVE (keeps both engines fed)
                for qi, (bb, it) in enumerate(quad):
                    T = tiles[bb, it]
                    if sim_compat or qi >= 2:
                        nc.vector.scalar_tensor_tensor(
                            T[:], T[:], NEG_SLOPE, T[:], AL.mult, AL.max
                        )
                    else:
                        nc.scalar.activation(T[:], T[:], ACTF.Lrelu, alpha=NEG_SLOPE)
                for bb, it in quad:
                    T = tiles[bb, it]
                    rs = mpool.tile([P, 1], F32, tag="rs")
                    E = mpool.tile([P, N], BF16, tag="E", bufs=4)
                    nc.scalar.activation(E[:], T[:], ACTF.Exp, accum_out=rs[:])
                    tiles["rs", bb, it] = rs
                    tiles["E", bb, it] = E
                for bb, it in quad:
                    E = tiles["E", bb, it]
                    rs = tiles["rs", bb, it]
                    xcol = xc_sb[:, it, bb : bb + 1]
                    EM = mpool.tile([P, N], BF16, tag="EM", bufs=4)
                    nc.vector.tensor_tensor(EM[:], E[:], masks[it][:], AL.mult)
                    recip = mpool.tile([P, 1], F32, tag="recip")
                    nc.vector.reciprocal(recip[:], rs[:])
                    coeff = mpool.tile([P, 1], BF16, tag="coeff")
                    nc.vector.tensor_scalar(coeff[:], recip[:], xcol, None, AL.mult)
                    for c in range(NKT):
                        sl = slice(c * 512, (c + 1) * 512)
                        nc.tensor.matmul(
                            accs[bb][:, sl], coeff[:], EM[:, sl],
                            start=(it == 0), stop=(it == ITILES - 1),
                        )
                for bb in bbs:
                    orow = mpool.tile([1, N], F32, tag="orow", bufs=2)
                    if bb % 2 == 0:
                        nc.scalar.copy(orow[:], accs[bb][:])
                    else:
                        nc.vector.tensor_copy(orow[:], accs[bb][:])
                    nc.sync.dma_start(ar_in[bb : bb + 1, :], orow[:])

                # ---- AllReduce each half as soon as it is done, so the
                # first collective hides behind the second half's compute
                if bp == 1:
                    nc.gpsimd.collective_compute(
                        "AllReduce",
                        AL.add,
                        replica_groups=groups,
                        ins=[ar_in[0 : B // 2, :].opt()],
                        outs=[ar_out0.opt()],
                    )
                    nc.sync.dma_start(out[0 : B // 2, :], ar_out0[:])
            nc.gpsimd.collective_compute(
                "AllReduce",
                AL.add,
                replica_groups=groups,
                ins=[ar_in[B // 2 :, :].opt()],
                outs=[ar_out1.opt()],
            )
            nc.sync.dma_start(out[B // 2 :, :], ar_out1[:])


def build_nc(sim_compat=False):
    nc = bacc.Bacc("TRN2", target_bir_lowering=False, num_devices=NCORES)
    adjc = nc.dram_tensor("adjc", [RPC, N], F32, kind="ExternalInput")
    xcT = nc.dram_tensor("xcT", [RPC, B], F32, kind="ExternalInput")
    xT = nc.dram_tensor("xT", [N, B], F32, kind="ExternalInput")
    wTc = nc.dram_tensor("wTc", [N, RPC], F32, kind="ExternalInput")
    bvec = nc.dram_tensor("bvec", [1, N], F32, kind="ExternalInput")
    medv = nc.dram_tensor("medv", [P, 1], F32, kind="ExternalInput")
    out = nc.dram_tensor("out", [B, N], F32, kind="ExternalOutput")
    with tile.TileContext(nc) as tc:
        _kernel(tc, adjc, xcT, xT, wTc, bvec, medv, out, sim_compat)
    nc.compile()
    return nc


def host_median(adj):
    """Lower median of softmax(adj, -1), float32, matching torch.median."""
    adj = np.asarray(adj, np.float32)
    m = adj.max(axis=1, keepdims=True)
    e = np.exp(adj - m, dtype=np.float32)
    a = (e / e.sum(axis=1, keepdims=True, dtype=np.float32)).astype(np.float32)
    flat = a.reshape(-1)
    kth = (flat.size - 1) // 2
    return np.partition(flat, kth)[kth]


def prepare_inputs(x, adj, W, b):
    x = np.ascontiguousarray(np.asarray(x, np.float32))
    adj = np.ascontiguousarray(np.asarray(adj, np.float32))
    W = np.ascontiguousarray(np.asarray(W, np.float32))
    b = np.ascontiguousarray(np.asarray(b, np.float32))
    med = host_median(adj)
    WT = np.ascontiguousarray(W.T)
    xT = np.ascontiguousarray(x.T)
    medv = np.full((P, 1), med, np.float32)
    bvec = np.ascontiguousarray(b.reshape(1, N))
    in_maps = []
    for c in range(NCORES):
        sl = slice(c * RPC, (c + 1) * RPC)
        in_maps.append(
            {
                "adjc": np.ascontiguousarray(adj[sl]),
                "xcT": np.ascontiguousarray(xT[sl]),
                "xT": xT,
                "wTc": np.ascontiguousarray(WT[:, sl]),
                "bvec": bvec,
                "medv": medv,
            }
        )
    return in_maps


_NC_CACHE = None


def kernel(x, adj, W, b):
    global _NC_CACHE, LAST_RESULTS
    if _NC_CACHE is None:
        _NC_CACHE = build_nc()
    in_maps = prepare_inputs(x, adj, W, b)
    res = run_bass_kernel_spmd(
        _NC_CACHE, in_maps, core_ids=list(range(NCORES)), trace=TRACE
    )
    LAST_RESULTS = res
    return np.asarray(res.results[0]["out"], np.float32)


# revision 29
# speedup vs baseline: 1.4692x; 1.0046x over previous
"""GAT-style message-passing kernel for Trainium2 (8 NeuronCores, Bass/Tile).

Reference computation (B=8, N=2048):
    a    = softmax(adj, -1); med = lower-median(a); mask = a > med
    w    = (x[:,:,None]*x[:,None,:]) @ W.T + b        # [B,N,N]
    w    = softmax(leaky_relu(w), -1) * mask
    out  = einsum('bi,bij->bj', x, w)

Key identity: w[b,i,k] = x[b,i]*y[b,k] + b[k] with y = x @ W.T (rank-1 +
bias), so no [B,N,N] matmul is needed; everything is fused elementwise
passes plus one weighted reduction:
    out[b,k] = sum_i (x[b,i]/rs[b,i]) * exp(lrelu(x[b,i]*y[b,k]+b[k])) * mask[i,k]
    rs[b,i]  = sum_k exp(lrelu(x[b,i]*y[b,k]+b[k]))

Sharding: rows i are split across the 8 cores (256 rows each, all 8
batches per core).  Each core computes its mask rows from its adj rows,
its shard of y = x@W.T (k-split, AllGather), the partial out over its i
rows, and an AllReduce produces the full output on every core.

The global lower-median of softmax(adj) (a 4M-element order statistic) is
computed on the host and passed in as a scalar; everything O(B*N*N) and
O(N*N) runs on device.
"""

import numpy as np

import concourse.bass as bass  # noqa: F401  (bass types via bacc/tile)
import concourse.mybir as mybir
import concourse.tile as tile
from concourse import bacc
from concourse.bass_utils import run_bass_kernel_spmd

N = 2048
B = 8
NCORES = 8
RPC = N // NCORES  # 256 rows (i) / cols (k) per core
P = 128
ITILES = RPC // P  # 2
NJT = N // P  # 16 j-tiles for the y matmul
NKT = N // 512  # psum-bank sized chunks of the free dim
NEG_SLOPE = 0.01
F32 = mybir.dt.float32
BF16 = mybir.dt.bfloat16
AL = mybir.AluOpType
ACTF = mybir.ActivationFunctionType

# test harness hooks
TRACE = False
LAST_RESULTS = None


def _kernel(tc, adjc, xcT, xT, wTc, bvec, medv, out, sim_compat):
    nc = tc.nc
    groups = [list(range(NCORES))]

    with (
        tc.tile_pool(name="const", bufs=1) as cpool,
        tc.tile_pool(name="dram", bufs=1, space="DRAM") as dpool,
    ):
        y_bcast = [
            cpool.tile([P, N], F32, tag=f"ybc{bb}", name=f"ybc{bb}") for bb in range(B)
        ]
        b_bcast = cpool.tile([P, N], F32)
        mask0 = cpool.tile([P, N], BF16, tag="mask0")
        mask1 = cpool.tile([P, N], BF16, tag="mask1")
        masks = [mask0, mask1]
        xc_sb = cpool.tile([P, ITILES, B], F32)
        med_sb = cpool.tile([P, 1], F32)



        # ---- y shard: y[:, kslice] = x @ W.T[:, kslice], then AllGather ----
        with (
            tc.tile_pool(name="ld", bufs=1) as ldpool,
            tc.tile_pool(name="ps_pre", bufs=1, space="PSUM") as pspre,
        ):
            # W first: the y-matmul -> AllGather -> broadcast chain is the
            # longest lead-in, so its DMA gets the queue head
            w_t = ldpool.tile([P, NJT, RPC], F32)
            nc.sync.dma_start(w_t[:], wTc[:].rearrange("(jt p) k -> p jt k", p=P))
            xT_t = ldpool.tile([P, NJT, B], F32)
            nc.sync.dma_start(xT_t[:], xT[:].rearrange("(jt p) b -> p jt b", p=P))
            nc.sync.dma_start(med_sb[:], medv[:])
            nc.sync.dma_start(
                xc_sb[:], xcT[:].rearrange("(it p) b -> p it b", p=P)
            )
            b_row = ldpool.tile([1, N], F32)
            nc.sync.dma_start(b_row[:], bvec[:])
            nc.gpsimd.partition_broadcast(b_bcast[:], b_row[:])

            y_ps = pspre.tile([B, RPC], F32)
            for jt in range(NJT):
                nc.tensor.matmul(
                    y_ps[:], xT_t[:, jt, :], w_t[:, jt, :],
                    start=(jt == 0), stop=(jt == NJT - 1),
                )
            y_part = ldpool.tile([B, RPC], F32)
            nc.scalar.copy(y_part[:], y_ps[:])

            yg_in = dpool.tile([B, RPC], F32)
            yg_out = dpool.tile([NCORES, B, RPC], F32, addr_space="Shared")
            nc.sync.dma_start(yg_in[:], y_part[:])
            nc.gpsimd.collective_compute(
                "AllGather",
                AL.bypass,
                replica_groups=groups,
                ins=[yg_in.opt()],
                outs=[yg_out.opt()],
            )
            # stage each y row on partition 0 (gpsimd queue, so the sync
            # queue never blocks on the collective), then broadcast it to
            # all partitions; the chain overlaps the main loop since each
            # batch's y_bcast tile has its own dependency
            yg_bview = yg_out[:].rearrange("r b k -> b r k")
            for bb in range(B):
                ystage = cpool.tile(
                    [1, NCORES, RPC], F32, tag="ystage", bufs=2, name=f"ystage{bb}"
                )
                nc.gpsimd.dma_start(ystage[:], yg_bview[bb : bb + 1])
                nc.gpsimd.partition_broadcast(y_bcast[bb][:], ystage[:])

        # ---- mask rows: softmax(adj_rows) > med  (no division needed) ----
        with tc.tile_pool(name="adjp", bufs=2) as apool:
            for it in range(ITILES):
                adj_t = apool.tile([P, N], F32, tag="adj")
                nc.sync.dma_start(adj_t[:], adjc[it * P : (it + 1) * P, :])
                nmax = apool.tile([P, 1], F32, tag="nmax")
                nc.vector.tensor_reduce(
                    nmax[:], adj_t[:], axis=mybir.AxisListType.X, op=AL.max,
                    negate=True,
                )
                eadj = apool.tile([P, N], F32, tag="eadj")
                rs_adj = apool.tile([P, 1], F32, tag="rsadj")
                nc.scalar.activation(
                    eadj[:], adj_t[:], ACTF.Exp, bias=nmax[:], scale=1.0,
                    accum_out=rs_adj[:],
                )
                thr = apool.tile([P, 1], F32, tag="thr")
                nc.vector.tensor_scalar(thr[:], rs_adj[:], med_sb[:], None, AL.mult)
                # mask = (exp(adj-max) > med*rowsum)  <=>  softmax(adj) > med
                nc.vector.tensor_scalar(masks[it][:], eadj[:], thr[:], None, AL.is_gt)

        # ---- main loop: groups of 4 (2 batches x 2 i-tiles), fused ----
        # grouping keeps the ACT engine on one function table at a time
        with (
            tc.tile_pool(name="main", bufs=6) as mpool,
            tc.tile_pool(name="ps_acc", bufs=2, space="PSUM") as psacc,
        ):
            ar_in = dpool.tile([B, N], F32)
            ar_out0 = dpool.tile([B // 2, N], F32, addr_space="Shared")
            ar_out1 = dpool.tile([B // 2, N], F32, addr_space="Shared")
            for bp in range(B // 2):
                bbs = (2 * bp, 2 * bp + 1)
                quad = [(bb, it) for bb in bbs for it in range(ITILES)]
                accs = {}
                for bb in bbs:
                    accs[bb] = psacc.tile([1, N], F32, tag="acc", name=f"acc{bb}")
                tiles = {}
                for bb, it in quad:
                    xcol = xc_sb[:, it, bb : bb + 1]
                    T = mpool.tile([P, N], F32, tag="T")
                    nc.vector.scalar_tensor_tensor(
                        T[:], y_bcast[bb][:], xcol, b_bcast[:], AL.mult, AL.add
                    )
                    tiles[bb, it] = T
                # lrelu: first half of the quad on ACT (one table load),
                # second half on DVE (keeps both engines fed)
                for qi, (bb, it) in enumerate(quad):
                    T = tiles[bb, it]
                    if sim_compat or qi >= 2:
                        nc.vector.scalar_tensor_tensor(
                            T[:], T[:], NEG_SLOPE, T[:], AL.mult, AL.max
                        )
                    else:
                        nc.scalar.activation(T[:], T[:], ACTF.Lrelu, alpha=NEG_SLOPE)
                for bb, it in quad:
                    T = tiles[bb, it]
                    rs = mpool.tile([P, 1], F32, tag="rs")
                    E = mpool.tile([P, N], BF16, tag="E", bufs=4)
                    nc.scalar.activation(E[:], T[:], ACTF.Exp, accum_out=rs[:])
                    tiles["rs", bb, it] = rs
                    tiles["E", bb, it] = E
                for bb, it in quad:
                    E = tiles["E", bb, it]
                    rs = tiles["rs", bb, it]
                    xcol = xc_sb[:, it, bb : bb + 1]
                    EM = mpool.tile([P, N], BF16, tag="EM", bufs=4)
                    nc.vector.tensor_tensor(EM[:], E[:], masks[it][:], AL.mult)
                    recip = mpool.tile([P, 1], F32, tag="recip")
                    nc.vector.reciprocal(recip[:], rs[:])
                    coeff = mpool.tile([P, 1], BF16, tag="coeff")
                    nc.vector.tensor_scalar(coeff[:], recip[:], xcol, None, AL.mult)
                    for c in range(NKT):
                        sl = slice(c * 512, (c + 1) * 512)
                        nc.tensor.matmul(
                            accs[bb][:, sl], coeff[:], EM[:, sl],
                            start=(it == 0), stop=(it == ITILES - 1),
                        )
                for bb in bbs:
                    orow = mpool.tile([1, N], F32, tag="orow", bufs=2)
                    if bb % 2 == 0:
                        nc.scalar.copy(orow[:], accs[bb][:])
                    else:
                        nc.vector.tensor_copy(orow[:], accs[bb][:])
                    nc.sync.dma_start(ar_in[bb : bb + 1, :], orow[:])

                # ---- AllReduce each half as soon as it is done, so the
                # first collective hides behind the second half's compute
                if bp == 1:
                    nc.gpsimd.collective_compute(
                        "AllReduce",
                        AL.add,
                        replica_groups=groups,
                        ins=[ar_in[0 : B // 2, :].opt()],
                        outs=[ar_out0.opt()],
                    )
                    nc.sync.dma_start(out[0 : B // 2, :], ar_out0[:])
            nc.gpsimd.collective_compute(
                "AllReduce",
                AL.add,
                replica_groups=groups,
                ins=[ar_in[B // 2 :, :].opt()],
                outs=[ar_out1.opt()],
            )
            nc.sync.dma_start(out[B // 2 :, :], ar_out1[:])


def build_nc(sim_compat=False):
    nc = bacc.Bacc("TRN2", target_bir_lowering=False, num_devices=NCORES)
    adjc = nc.dram_tensor("adjc", [RPC, N], F32, kind="ExternalInput")
    xcT = nc.dram_tensor("xcT", [RPC, B], F32, kind="ExternalInput")
    xT = nc.dram_tensor("xT", [N, B], F32, kind="ExternalInput")
    wTc = nc.dram_tensor("wTc", [N, RPC], F32, kind="ExternalInput")
    bvec = nc.dram_tensor("bvec", [1, N], F32, kind="ExternalInput")
    medv = nc.dram_tensor("medv", [P, 1], F32, kind="ExternalInput")
    out = nc.dram_tensor("out", [B, N], F32, kind="ExternalOutput")
    with tile.TileContext(nc) as tc:
        _kernel(tc, adjc, xcT, xT, wTc, bvec, medv, out, sim_compat)
    nc.compile()
    return nc


def host_median(adj):
    """Lower median of softmax(adj, -1), float32, matching torch.median."""
    adj = np.asarray(adj, np.float32)
    m = adj.max(axis=1, keepdims=True)
    e = np.exp(adj - m, dtype=np.float32)
    a = (e / e.sum(axis=1, keepdims=True, dtype=np.float32)).astype(np.float32)
    flat = a.reshape(-1)
    kth = (flat.size - 1) // 2
    return np.partition(flat, kth)[kth]


def prepare_inputs(x, adj, W, b):
    x = np.ascontiguousarray(np.asarray(x, np.float32))
    adj = np.ascontiguousarray(np.asarray(adj, np.float32))
    W = np.ascontiguousarray(np.asarray(W, np.float32))
    b = np.ascontiguousarray(np.asarray(b, np.float32))
    med = host_median(adj)
    WT = np.ascontiguousarray(W.T)
    xT = np.ascontiguousarray(x.T)
    medv = np.full((P, 1), med, np.float32)
    bvec = np.ascontiguousarray(b.reshape(1, N))
    in_maps = []
    for c in range(NCORES):
        sl = slice(c * RPC, (c + 1) * RPC)
        in_maps.append(
            {
                "adjc": np.ascontiguousarray(adj[sl]),
                "xcT": np.ascontiguousarray(xT[sl]),
                "xT": xT,
                "wTc": np.ascontiguousarray(WT[:, sl]),
                "bvec": bvec,
                "medv": medv,
            }
        )
    return in_maps


_NC_CACHE = None


def kernel(x, adj, W, b):
    global _NC_CACHE, LAST_RESULTS
    if _NC_CACHE is None:
        _NC_CACHE = build_nc()
    in_maps = prepare_inputs(x, adj, W, b)
    res = run_bass_kernel_spmd(
        _NC_CACHE, in_maps, core_ids=list(range(NCORES)), trace=TRACE
    )
    LAST_RESULTS = res
    return np.asarray(res.results[0]["out"], np.float32)
